# revision 1
# baseline (speedup 1.0000x reference)
"""ArcticMoE top-2 MoE kernel for 8 Trainium2 NeuronCores.

Strategy (expert-parallel, sparse):
  - Host: router (logits -> softmax -> top-k -> renorm), per-expert token
    gather, transpose activations/weights into PE-friendly layouts.
  - Device (SPMD, 8 cores, 2 expert slots/core): for each owned expert compute
    y.T = w2 @ (silu(g.T) * u.T) where [g.T; u.T] = w13 @ x_e.T, fp16 matmuls (speed-identical to bf16, 8x better mantissa),
    feature dim on partitions, tokens on the moving/free axis.
  - Host: unweighted expert outputs scatter-added back with routing coefs.

The reference computes every expert densely; only top-2 contribute, so the
sparse form does 1/8th the FLOPs. Experts are assigned to slots by load
(8 biggest -> slot0 with capacity C0, 8 smallest -> slot1 with C1) so the
SPMD graph pads as little as possible. No collectives: each core's work is
independent and the combine happens on host.

PE efficiency: the k-loop loads each weight k-slice once and issues the
matmuls for every token block back-to-back; the duplicate Ldweights bass
emits for the second block are stripped from the BIR before compile.
"""

import numpy as np

T, H, I, E = 4096, 2048, 2048, 16
N_CORES = 8
EPC = E // N_CORES   # expert slots per core
KT = H // 128        # k-tiles over H (matmul 1 contraction)
MT1 = 2 * I // 128   # m-tiles over 2I (matmul 1 output rows)
IT = I // 128        # k-tiles over I (matmul 2 contraction)
MT2 = H // 128       # m-tiles over H (matmul 2 output rows)

_CACHE = {}
LAST_EXEC_NS = None  # exec_time_ns from the last run, when profiling is available


def _pad(v, g):
    return max(g, -(-v // g) * g)


def _blocks_of(C):
    return [C] if C <= 512 else [C // 2, C // 2]


def _dedup_ldweights(nc):
    """Remove InstLdweights that reload the identical weights AP as the
    previous Ldweights in the same basic block with only Matmults between."""
    removed = 0
    for bb in nc.m.functions[0].blocks:
        insts = bb.instructions
        keep = []
        last_key = None
        for inst in insts:
            tn = type(inst).__name__
            if tn == "InstLdweights":
                key = (
                    str(inst.ins[0]), str(inst.tile_position),
                    str(inst.tile_size), str(inst.perf_mode),
                    bool(inst.is_transpose),
                )
                if key == last_key and not inst.has_wait() and not inst.has_update():
                    removed += 1
                    continue
                last_key = key
            elif tn == "InstMatmult":
                pass
            else:
                last_key = None
            keep.append(inst)
        if len(keep) != len(insts):
            bb.instructions = keep
    return removed


def _batch_sem_incs(nc):
    """Collapse runs of per-instruction `sem-inc` updates on the PE stream
    into one `sem-add-imm` on the last instruction of each run.

    Tile has every matmul bump the PE engine semaphore; each bump is a
    serialized EVT_SEM register write (~26 ns), ~80 us across 3k matmuls.
    Consumers only ever need accumulation-chain-final ticks, so deferring
    intermediate bumps to the next run boundary is semantics-preserving:
    a run never extends past a PE instruction that carries a wait or a
    non-inc update, the summed value is attached at the boundary (so the
    semaphore total after any wait-carrying instruction is unchanged), and
    PE always reaches the boundary without blocking (no waits inside a
    run), so no deadlock can be introduced.
    """
    import concourse.mybir as mybir

    batched = 0
    for bb in nc.m.functions[0].blocks:
        pending = 0
        last_inc = None
        cur = None

        def flush():
            nonlocal pending, last_inc, cur, batched
            if last_inc is not None and pending > 0:
                si = last_inc.sync_info
                u = mybir.SyncUpdate(
                    sync_type=cur[0], id=cur[1], ant_name=cur[2],
                    update_mode="sem-add-imm", update_value=pending,
                    update_reg=None,
                )
                last_inc.sync_info = mybir.SyncInfo(
                    on_wait=list(si.on_wait) if si is not None else [],
                    on_update=[u],
                )
                batched += pending - 1
            pending = 0
            last_inc = None
            cur = None

        for inst in bb.instructions:
            if getattr(inst, "engine", None) != mybir.EngineType.PE:
                continue
            si = inst.sync_info
            waits = list(si.on_wait) if si is not None else []
            ups = list(si.on_update) if si is not None else []
            if waits:
                flush()
            strip = (
                len(ups) == 1
                and ups[0].sync_type == "semaphore"
                and ups[0].update_mode == "sem-inc"
                and (ups[0].update_value in (None, 1))
                and ups[0].update_reg is None
            )
            if strip:
                key = (ups[0].sync_type, ups[0].id, ups[0].ant_name)
                if cur is not None and key != cur:
                    flush()
                inst.sync_info = mybir.SyncInfo(on_wait=waits, on_update=[])
                cur = key
                pending += 1
                last_inc = inst
            elif ups:
                flush()
        flush()
    return batched


def _build(caps, R=None, loop_kwargs=None):
    """Build + compile the per-core Bass program for slot capacities `caps`.

    R: when set, wrap the whole body in a hardware For_i loop that repeats it
    R times (used only by the timing harness; kernel() always passes None).
    loop_kwargs: extra For_i options for that harness loop (e.g.
    staggered_reset) — irrelevant to the real single-shot kernel.
    """
    import concourse.mybir as mybir
    from concourse import bacc
    from concourse.tile import TileContext

    F32 = mybir.dt.float32
    F16 = mybir.dt.float16
    Silu = mybir.ActivationFunctionType.Silu

    nc = bacc.Bacc("TRN2", target_bir_lowering=False, debug=False, num_devices=N_CORES)
    xts_d, outs_d = [], []
    for s, C in enumerate(caps):
        xts_d.append(nc.dram_tensor(f"xt{s}", [128, KT, C], F16, kind="ExternalInput"))
        outs_d.append(nc.dram_tensor(f"out{s}", [MT2, 128, C], F32, kind="ExternalOutput"))
    w1 = nc.dram_tensor("w1", [EPC, MT1, 128, KT, 128], F16, kind="ExternalInput")
    w2 = nc.dram_tensor("w2", [EPC, MT2, 128, IT, 128], F16, kind="ExternalInput")

    Cmax = max(caps)
    with TileContext(nc) as tc:
        with (
            tc.tile_pool(name="x", bufs=2) as xpool,
            tc.tile_pool(name="h", bufs=2) as hpool,
            tc.tile_pool(name="w", bufs=8) as wpool,
            tc.tile_pool(name="y", bufs=3) as ypool,
            tc.tile_pool(name="ps", bufs=8, space="PSUM") as pspool,
        ):
            def mm_group(dst_tiles, wtile, src, bss, KTN):
                for k in range(KTN):
                    for ps, bs in zip(dst_tiles, bss):
                        nc.tensor.matmul(
                            ps[:], wtile[:, k], src[:, k, bs],
                            start=(k == 0), stop=(k == KTN - 1),
                        )

            def body():
                for s, C in enumerate(caps):
                    bls = _blocks_of(C)
                    bss = []
                    off = 0
                    for CB in bls:
                        bss.append(slice(off, off + CB))
                        off += CB
                    xts = xpool.tile([128, KT, Cmax], F16, tag="x", name="xts")[:, :, :C]
                    # front xts half, then the j0 weights and the first w2
                    # tile, then the back half: the j0 chains start as soon
                    # as possible and neither phase head waits on a DMA
                    nc.sync.dma_start(xts[:, 0:8, :], xts_d[s][:, 0:8, :])
                    w_pre = {}
                    for nm, src in (("wg", w1[s, 0]), ("wu", w1[s, IT]),
                                    ("w2h", w2[s, 0])):
                        t = wpool.tile([128, KT, 128], F16, tag="w", name=nm)
                        nc.sync.dma_start(t[:], src)
                        w_pre[nm] = t
                    nc.sync.dma_start(xts[:, 8:KT, :], xts_d[s][:, 8:KT, :])
                    hbuf = hpool.tile([128, IT, Cmax], F16, tag="h", name="hbuf")[:, :, :C]
                    # ---- matmul 1 (w13 @ x.T) fused with SiluAndMul ----
                    for j in range(IT):
                        if j == 0:
                            wg, wu = w_pre["wg"], w_pre["wu"]
                        else:
                            wg = wpool.tile([128, KT, 128], F16, tag="w")
                            nc.sync.dma_start(wg[:], w1[s, j])
                            wu = wpool.tile([128, KT, 128], F16, tag="w")
                            nc.sync.dma_start(wu[:], w1[s, j + IT])
                        psgs = [
                            pspool.tile([128, 512], F32, tag="ps", name="psg")[:, :CB]
                            for CB in bls
                        ]
                        mm_group(psgs, wg, xts, bss, KT)
                        psus = [
                            pspool.tile([128, 512], F32, tag="ps", name="psu")[:, :CB]
                            for CB in bls
                        ]
                        mm_group(psus, wu, xts, bss, KT)
                        for b in range(len(bls)):
                            hs = hbuf[:, j, bss[b]]
                            nc.scalar.activation(hs, psgs[b][:], Silu)
                            nc.vector.tensor_mul(hs, hs, psus[b][:])
                    # ---- matmul 2 (w2 @ h.T) ----
                    for m in range(MT2):
                        if m == 0:
                            w2t = w_pre["w2h"]
                        else:
                            w2t = wpool.tile([128, IT, 128], F16, tag="w")
                            nc.sync.dma_start(w2t[:], w2[s, m])
                        yt = ypool.tile([128, Cmax], F32, tag="y", name="yt")[:, :C]
                        pss = [
                            pspool.tile([128, 512], F32, tag="ps", name="pso")[:, :CB]
                            for CB in bls
                        ]
                        mm_group(pss, w2t, hbuf, bss, IT)
                        for b in range(len(bls)):
                            nc.vector.tensor_copy(yt[:, bss[b]], pss[b][:])
                        nc.sync.dma_start(outs_d[s][m], yt[:])

            if R is None:
                body()
            else:
                with tc.For_i(0, R, 1, **(loop_kwargs or {})):
                    body()
    _dedup_ldweights(nc)
    _batch_sem_incs(nc)
    nc.compile()
    return nc


def _get_nc(caps):
    if caps not in _CACHE:
        _CACHE[caps] = _build(caps)
    return _CACHE[caps]


def _route(x, gate_w, top_k):
    """Replicate the reference router on host. Returns (order [T,k], coefs [T,k])."""
    logits = x @ gate_w.T
    m = logits.max(-1, keepdims=True)
    ex = np.exp(logits - m)
    scores = ex / ex.sum(-1, keepdims=True)
    order = np.argsort(-scores, axis=-1)[:, :top_k]
    tw = np.take_along_axis(scores, order, -1)
    if top_k > 1:
        tw = tw / tw.sum(-1, keepdims=True)
    return order.astype(np.int64), tw.astype(np.float32)


def kernel(x, gate_w, ws, w2s, top_k):
    from concourse.bass_utils import run_bass_kernel_spmd

    f16 = np.float16
    x = np.ascontiguousarray(np.asarray(x, dtype=np.float32))
    gate_w = np.asarray(gate_w, dtype=np.float32)
    ws = np.asarray(ws, dtype=np.float32)
    w2s = np.asarray(w2s, dtype=np.float32)
    top_k = int(np.asarray(top_k))

    order, tw = _route(x, gate_w, top_k)

    ids = [np.nonzero((order == e).any(-1))[0] for e in range(E)]
    counts = np.array([len(i) for i in ids])

    # slot assignment: 8 biggest experts -> slot0, 8 smallest -> slot1
    perm = np.argsort(-counts, kind="stable")
    slot_experts = [perm[:N_CORES], perm[N_CORES:]]
    # pad-8 keeps DMA lines 16B-aligned; measured faster than pad-64 (fewer
    # streamed columns dominates) now that the PE stream is the bottleneck
    caps = tuple(_pad(int(counts[se].max()), 8) for se in slot_experts)
    assert caps[0] <= 1024 and caps[1] <= 1024, caps

    nc = _get_nc(caps)

    # weights, pre-transposed + tiled + bf16, per expert
    # w13 lhsT tile layout: w1h[e, m, p, k, c] = ws[e, m*128+c, k*128+p]
    in_maps = []
    for c in range(N_CORES):
        owned = [int(slot_experts[s][c]) for s in range(EPC)]
        m = {
            "w1": np.ascontiguousarray(
                ws[owned].reshape(EPC, MT1, 128, KT, 128).transpose(0, 1, 4, 3, 2)
            ).astype(f16),
            "w2": np.ascontiguousarray(
                w2s[owned].reshape(EPC, MT2, 128, IT, 128).transpose(0, 1, 4, 3, 2)
            ).astype(f16),
        }
        for s, e in enumerate(owned):
            C = caps[s]
            xt_host = np.zeros((128, KT, C), f16)
            xe = x[ids[e]]  # [n_e, H]
            # xt[p, k, t] = xe[t, k*128+p]
            xt_host[:, :, : len(ids[e])] = (
                xe.T.reshape(KT, 128, -1).transpose(1, 0, 2).astype(f16)
            )
            m[f"xt{s}"] = xt_host
        in_maps.append(m)

    try:
        res = run_bass_kernel_spmd(nc, in_maps, core_ids=list(range(N_CORES)))
    except ModuleNotFoundError:
        # BASS_TRACE set but this axon client has no NTFF profile hook
        import os

        os.environ["BASS_NEVER_TRACE"] = "1"
        res = run_bass_kernel_spmd(nc, in_maps, core_ids=list(range(N_CORES)))
    global LAST_EXEC_NS
    LAST_EXEC_NS = res.exec_time_ns

    out = np.zeros((T, H), np.float32)
    for c in range(N_CORES):
        for s in range(EPC):
            e = int(slot_experts[s][c])
            n_e = len(ids[e])
            if n_e == 0:
                continue
            yts = res.results[c][f"out{s}"]  # [MT2, 128, C]
            ye = yts.transpose(2, 0, 1).reshape(caps[s], H)[:n_e]
            sel = order[ids[e]] == e  # [n_e, k]
            coef = (tw[ids[e]] * sel).sum(-1).astype(np.float32)
            out[ids[e]] += coef[:, None] * ye
    return out



# revision 3
# speedup vs baseline: 1.0773x; 1.0773x over previous
"""ArcticMoE top-2 MoE kernel for 8 Trainium2 NeuronCores.

Strategy (expert-parallel, sparse):
  - Host: router (logits -> softmax -> top-k -> renorm), per-expert token
    gather, transpose activations/weights into PE-friendly layouts.
  - Device (SPMD, 8 cores, 2 expert slots/core): for each owned expert compute
    y.T = w2 @ (silu(g.T) * u.T) where [g.T; u.T] = w13 @ x_e.T, fp16 matmuls
    (speed-identical to bf16, 8x better mantissa), feature dim on partitions,
    tokens on the moving/free axis.
  - Host: unweighted expert outputs scatter-added back with routing coefs.

The reference computes every expert densely; only top-2 contribute, so the
sparse form does 1/8th the FLOPs. Experts are assigned to slots by load
(8 biggest -> slot0 with capacity C0, 8 smallest -> slot1 with C1) so the
SPMD graph pads as little as possible. No collectives: each core's work is
independent and the combine happens on host.

v4 structure (HW-measured on the axon trn2 cores):
  - weight DMA batched: one 1MB transfer per j carries both the gate and up
    k-slabs (w1 host layout [EPC, IT, 2, 128, KT, 128]);
  - outputs fp16 in [128, MT2, C] layout, written with one DMA per 2 m-tiles
    (halves output traffic; DVE psum->SBUF copies run in 16-bit mode);
  - slot1's activations/first weights prefetch during slot0's mm2 so the
    PE never waits at the slot boundary;
  - boot order: first 4 k-tiles of x, first w1 slab, rest of x, then w2
    head — the first matmul chain can start after ~1.3MB of DMA.
  - flat instruction stream: tc.For_i bodies measured ~30us SLOWER than the
    fully unrolled stream (loop-boundary drains), so everything is unrolled.

PE efficiency notes: LDW+MM(N) streams at ~N/2.4GHz + ~20-27ns fixed per MM
(HW-measured; LDWEIGHTS itself pipelines for free). The per-MM overhead and
the ~1 col/cycle fp16 stream rate put the PE floor for this decomposition at
~400us/core; fp8 DoubleRow (2 cols/cycle) fails the 2e-2 accuracy gate
(measured 6.6% end-to-end), so fp16 it is.

The duplicate Ldweights bass emits for the second 2-block matmul are
stripped from the BIR before compile; per-matmul semaphore bumps are
batched into one sem-add-imm per run boundary.
"""

import numpy as np

T, H, I, E = 4096, 2048, 2048, 16
N_CORES = 8
EPC = E // N_CORES   # expert slots per core
KT = H // 128        # k-tiles over H (matmul 1 contraction)
MT1 = 2 * I // 128   # m-tiles over 2I (matmul 1 output rows)
IT = I // 128        # k-tiles over I (matmul 2 contraction)
MT2 = H // 128       # m-tiles over H (matmul 2 output rows)

_CACHE = {}
LAST_EXEC_NS = None  # exec_time_ns from the last run, when profiling is available


def _pad(v, g):
    return max(g, -(-v // g) * g)


def _blocks_of(C):
    return [C] if C <= 512 else [C // 2, C // 2]


def _dedup_ldweights(nc):
    """Remove InstLdweights that reload the identical weights AP as the
    previous Ldweights in the same basic block with only Matmults between."""
    removed = 0
    for bb in nc.m.functions[0].blocks:
        insts = bb.instructions
        keep = []
        last_key = None
        for inst in insts:
            tn = type(inst).__name__
            if tn == "InstLdweights":
                key = (
                    str(inst.ins[0]), str(inst.tile_position),
                    str(inst.tile_size), str(inst.perf_mode),
                    bool(inst.is_transpose),
                )
                if key == last_key and not inst.has_wait() and not inst.has_update():
                    removed += 1
                    continue
                last_key = key
            elif tn == "InstMatmult":
                pass
            else:
                last_key = None
            keep.append(inst)
        if len(keep) != len(insts):
            bb.instructions = keep
    return removed


def _batch_sem_incs(nc):
    """Collapse runs of per-instruction `sem-inc` updates on the PE stream
    into one `sem-add-imm` on the last instruction of each run.

    Tile has every matmul bump the PE engine semaphore; each bump is a
    serialized EVT_SEM register write (~26 ns), ~80 us across 3k matmuls.
    Consumers only ever need accumulation-chain-final ticks, so deferring
    intermediate bumps to the next run boundary is semantics-preserving:
    a run never extends past a PE instruction that carries a wait or a
    non-inc update, the summed value is attached at the boundary (so the
    semaphore total after any wait-carrying instruction is unchanged), and
    PE always reaches the boundary without blocking (no waits inside a
    run), so no deadlock can be introduced.
    """
    import concourse.mybir as mybir

    batched = 0
    for bb in nc.m.functions[0].blocks:
        pending = 0
        last_inc = None
        cur = None

        def flush():
            nonlocal pending, last_inc, cur, batched
            if last_inc is not None and pending > 0:
                si = last_inc.sync_info
                u = mybir.SyncUpdate(
                    sync_type=cur[0], id=cur[1], ant_name=cur[2],
                    update_mode="sem-add-imm", update_value=pending,
                    update_reg=None,
                )
                last_inc.sync_info = mybir.SyncInfo(
                    on_wait=list(si.on_wait) if si is not None else [],
                    on_update=[u],
                )
                batched += pending - 1
            pending = 0
            last_inc = None
            cur = None

        for inst in bb.instructions:
            if getattr(inst, "engine", None) != mybir.EngineType.PE:
                continue
            si = inst.sync_info
            waits = list(si.on_wait) if si is not None else []
            ups = list(si.on_update) if si is not None else []
            if waits:
                flush()
            strip = (
                len(ups) == 1
                and ups[0].sync_type == "semaphore"
                and ups[0].update_mode == "sem-inc"
                and (ups[0].update_value in (None, 1))
                and ups[0].update_reg is None
            )
            if strip:
                key = (ups[0].sync_type, ups[0].id, ups[0].ant_name)
                if cur is not None and key != cur:
                    flush()
                inst.sync_info = mybir.SyncInfo(on_wait=waits, on_update=[])
                cur = key
                pending += 1
                last_inc = inst
            elif ups:
                flush()
        flush()
    return batched


def _build(caps, R=None, loop_kwargs=None):
    """Build + compile the per-core Bass program for slot capacities `caps`.

    R: when set, wrap the whole body in a hardware For_i loop that repeats it
    R times (used only by the timing harness; kernel() always passes None).
    """
    import concourse.mybir as mybir
    from concourse import bacc
    from concourse.tile import TileContext

    F32 = mybir.dt.float32
    F16 = mybir.dt.float16
    Silu = mybir.ActivationFunctionType.Silu

    nc = bacc.Bacc("TRN2", target_bir_lowering=False, debug=False, num_devices=N_CORES)
    xts_d, outs_d = [], []
    for s, C in enumerate(caps):
        xts_d.append(nc.dram_tensor(f"xt{s}", [128, KT, C], F16, kind="ExternalInput"))
        outs_d.append(nc.dram_tensor(f"out{s}", [128, MT2, C], F16, kind="ExternalOutput"))
    w1 = nc.dram_tensor("w1", [EPC, IT, 128, 2, KT, 128], F16, kind="ExternalInput")
    w2 = nc.dram_tensor("w2", [EPC, MT2, 128, IT, 128], F16, kind="ExternalInput")

    Cmax = max(caps)
    with TileContext(nc) as tc:
        with (
            tc.tile_pool(name="x", bufs=2) as xpool,
            tc.tile_pool(name="h", bufs=2) as hpool,
            tc.tile_pool(name="w", bufs=6) as wpool,
            tc.tile_pool(name="w2p", bufs=8) as w2pool,
            tc.tile_pool(name="y", bufs=3) as ypool,
            tc.tile_pool(name="ps", bufs=8, space="PSUM") as pspool,
        ):
            def mm_group(dst_tiles, wtile, src, bss, KTN):
                for k in range(KTN):
                    for ps, bs in zip(dst_tiles, bss):
                        nc.tensor.matmul(
                            ps[:], wtile[:, k], src[:, k, bs],
                            start=(k == 0), stop=(k == KTN - 1),
                        )

            def body():
                def boot_slot(s, C, first):
                    xts = xpool.tile([128, KT, Cmax], F16, tag="x", name="xts")[:, :, :C]
                    if first:
                        nc.sync.dma_start(xts[:, 0:4, :], xts_d[s][:, 0:4, :])
                    wgu0 = wpool.tile([128, 2, KT, 128], F16, tag="w", name="wgu0")
                    nc.sync.dma_start(wgu0[:], w1[s, 0])
                    if first:
                        nc.sync.dma_start(xts[:, 4:KT, :], xts_d[s][:, 4:KT, :])
                    else:
                        nc.sync.dma_start(xts[:, 0:KT, :], xts_d[s][:, 0:KT, :])
                    w2h = w2pool.tile([128, IT, 128], F16, tag="w2", name="w2h")
                    nc.sync.dma_start(w2h[:], w2[s, 0])
                    return xts, {"wgu": wgu0, "w2h": w2h}

                def mm1_slot(s, C, xts, w_pre, bls, bss):
                    hbuf = hpool.tile([128, IT, Cmax], F16, tag="h", name="hbuf")[:, :, :C]
                    for j in range(IT):
                        if j == 0:
                            wgu = w_pre["wgu"]
                        else:
                            wgu = wpool.tile([128, 2, KT, 128], F16, tag="w")
                            nc.sync.dma_start(wgu[:], w1[s, j])
                        psgs = [
                            pspool.tile([128, 512], F32, tag="ps", name="psg")[:, :CB]
                            for CB in bls
                        ]
                        mm_group(psgs, wgu[:, 0], xts, bss, KT)
                        psus = [
                            pspool.tile([128, 512], F32, tag="ps", name="psu")[:, :CB]
                            for CB in bls
                        ]
                        mm_group(psus, wgu[:, 1], xts, bss, KT)
                        for b in range(len(bls)):
                            hs = hbuf[:, j, bss[b]]
                            nc.scalar.activation(hs, psgs[b][:], Silu)
                            nc.vector.tensor_mul(hs, hs, psus[b][:])
                    return hbuf

                def mm2_slot(s, C, hbuf, w_pre, bls, bss):
                    for m2 in range(MT2 // 2):
                        yt = ypool.tile([128, 2, Cmax], F16, tag="y", name="yt")[:, :, :C]
                        for mm in range(2):
                            m = m2 * 2 + mm
                            if m == 0:
                                w2t = w_pre["w2h"]
                            else:
                                w2t = w2pool.tile([128, IT, 128], F16, tag="w2")
                                nc.sync.dma_start(w2t[:], w2[s, m])
                            pss = [
                                pspool.tile([128, 512], F32, tag="ps", name="pso")[:, :CB]
                                for CB in bls
                            ]
                            mm_group(pss, w2t, hbuf, bss, IT)
                            for b in range(len(bls)):
                                nc.vector.tensor_copy(yt[:, mm, bss[b]], pss[b][:])
                        nc.sync.dma_start(outs_d[s][:, m2 * 2 : m2 * 2 + 2, :], yt[:])

                geom = {}
                for s, C in enumerate(caps):
                    bls = _blocks_of(C)
                    bss, off = [], 0
                    for CB in bls:
                        bss.append(slice(off, off + CB))
                        off += CB
                    geom[s] = (bls, bss)

                xts0, pre0 = boot_slot(0, caps[0], True)
                hbuf0 = mm1_slot(0, caps[0], xts0, pre0, *geom[0])
                xts1, pre1 = boot_slot(1, caps[1], False)
                mm2_slot(0, caps[0], hbuf0, pre0, *geom[0])
                hbuf1 = mm1_slot(1, caps[1], xts1, pre1, *geom[1])
                mm2_slot(1, caps[1], hbuf1, pre1, *geom[1])

            if R is None:
                body()
            else:
                with tc.For_i(0, R, 1, **(loop_kwargs or {})):
                    body()
    _dedup_ldweights(nc)
    _batch_sem_incs(nc)
    nc.compile()
    return nc


def _get_nc(caps):
    if caps not in _CACHE:
        _CACHE[caps] = _build(caps)
    return _CACHE[caps]


def _route(x, gate_w, top_k):
    """Replicate the reference router on host. Returns (order [T,k], coefs [T,k])."""
    logits = x @ gate_w.T
    m = logits.max(-1, keepdims=True)
    ex = np.exp(logits - m)
    scores = ex / ex.sum(-1, keepdims=True)
    order = np.argsort(-scores, axis=-1)[:, :top_k]
    tw = np.take_along_axis(scores, order, -1)
    if top_k > 1:
        tw = tw / tw.sum(-1, keepdims=True)
    return order.astype(np.int64), tw.astype(np.float32)


def kernel(x, gate_w, ws, w2s, top_k):
    from concourse.bass_utils import run_bass_kernel_spmd

    f16 = np.float16
    x = np.ascontiguousarray(np.asarray(x, dtype=np.float32))
    gate_w = np.asarray(gate_w, dtype=np.float32)
    ws = np.asarray(ws, dtype=np.float32)
    w2s = np.asarray(w2s, dtype=np.float32)
    top_k = int(np.asarray(top_k))

    order, tw = _route(x, gate_w, top_k)

    ids = [np.nonzero((order == e).any(-1))[0] for e in range(E)]
    counts = np.array([len(i) for i in ids])

    # slot assignment: 8 biggest experts -> slot0, 8 smallest -> slot1
    perm = np.argsort(-counts, kind="stable")
    slot_experts = [perm[:N_CORES], perm[N_CORES:]]
    # pad-8 keeps DMA lines 16B-aligned; measured faster than pad-64 (fewer
    # streamed columns dominates) now that the PE stream is the bottleneck
    caps = tuple(_pad(int(counts[se].max()), 8) for se in slot_experts)
    assert caps[0] <= 1024 and caps[1] <= 1024, caps

    nc = _get_nc(caps)

    # weights, pre-transposed + tiled + fp16, per expert
    # w13 lhsT tile layout: [e, j, p, {g,u}, k, c]:
    #   [e, j, p, 0, k, c] = ws[e, j*128+c,        k*128+p]   (gate rows)
    #   [e, j, p, 1, k, c] = ws[e, (j+IT)*128+c,   k*128+p]   (up rows)
    in_maps = []
    for c in range(N_CORES):
        owned = [int(slot_experts[s][c]) for s in range(EPC)]
        w1b = np.ascontiguousarray(
            ws[owned].reshape(EPC, MT1, 128, KT, 128).transpose(0, 1, 4, 3, 2)
        ).astype(f16)  # [EPC, MT1, 128p, KT, 128c] -> m-major
        w1i = np.stack((w1b[:, :IT], w1b[:, IT:]), axis=3)  # [EPC, IT, 128, 2, KT, 128]
        m = {
            "w1": np.ascontiguousarray(w1i),
            "w2": np.ascontiguousarray(
                w2s[owned].reshape(EPC, MT2, 128, IT, 128).transpose(0, 1, 4, 3, 2)
            ).astype(f16),
        }
        for s, e in enumerate(owned):
            C = caps[s]
            xt_host = np.zeros((128, KT, C), f16)
            xe = x[ids[e]]  # [n_e, H]
            # xt[p, k, t] = xe[t, k*128+p]
            xt_host[:, :, : len(ids[e])] = (
                xe.T.reshape(KT, 128, -1).transpose(1, 0, 2).astype(f16)
            )
            m[f"xt{s}"] = xt_host
        in_maps.append(m)

    try:
        res = run_bass_kernel_spmd(nc, in_maps, core_ids=list(range(N_CORES)))
    except ModuleNotFoundError:
        # BASS_TRACE set but this axon client has no NTFF profile hook
        import os

        os.environ["BASS_NEVER_TRACE"] = "1"
        res = run_bass_kernel_spmd(nc, in_maps, core_ids=list(range(N_CORES)))
    global LAST_EXEC_NS
    LAST_EXEC_NS = res.exec_time_ns

    out = np.zeros((T, H), np.float32)
    for c in range(N_CORES):
        for s in range(EPC):
            e = int(slot_experts[s][c])
            n_e = len(ids[e])
            if n_e == 0:
                continue
            yts = res.results[c][f"out{s}"]  # [128, MT2, C] f16
            ye = yts.transpose(2, 1, 0).reshape(caps[s], H)[:n_e].astype(np.float32)
            sel = order[ids[e]] == e  # [n_e, k]
            coef = (tw[ids[e]] * sel).sum(-1).astype(np.float32)
            out[ids[e]] += coef[:, None] * ye
    return out
